# revision 11
# baseline (speedup 1.0000x reference)
"""Trainium2 Bass kernel for nn_DecoderBlock (dense transformer decoder block).

Sharding: data-parallel over batch N=8 -> one batch element per NeuronCore.
Zero collectives; weights replicated to every core.

Per-core computation (K=1024 tokens, M=1024 emb, H=8 heads, DH=128, FF=4096):
  a1 = MHA(dec, dec);  x1 = LN(dec + a1)
  a2 = MHA(x1, enc);   x2 = LN(x1 + a2)
  ff = relu(x2 @ W1.T) @ W2.T;  out = LN(x2 + ff)

v2 design notes:
  - All matmul operands are bf16 (same 1 cyc/row PE rate as fp32r, half the
    DMA/SBUF). Accumulation, LN, residual math stay fp32. Measured numpy
    rel-err for this scheme: 5.2e-3.
  - Attention is software-pipelined at (head, query-half) "unit" granularity
    with a one-unit lag: scores of unit u interleave with denominator/PV
    matmuls of unit u-1, so the PE never waits on the Scalar-engine exp.
  - Softmax denominator via ones-matmul broadcast; reciprocal via the fast
    custom DVE op; normalization folded into the ycat multiply.
  - K projections + V projections of the *next* attention are emitted as
    fillers inside the current Wo+LN block, so those phases stay PE-dense.
  - FFN: ft-outer FFN1 (W1 loaded once), kt-outer FFN2 (W2 resident in SBUF,
    LN of token tile kt overlaps matmuls of kt+1).
"""
import sys

sys.path.insert(0, "/opt/trn_rl_repo")

import numpy as np
import ml_dtypes

# antenv.axon_hooks shim (needed only if BASS_TRACE is set; the agent image's
# read-only antenv package lacks this module).
try:
    from antenv import axon_hooks as _ah  # noqa: F401
except ImportError:
    import types as _types

    _h = _types.ModuleType("antenv.axon_hooks")
    _h._HOOK = None

    def _get_hook():
        if _h._HOOK is None:
            try:
                from trn_agent_boot.trn_boot import _ntff_profile_via_ctypes

                _h._HOOK = _ntff_profile_via_ctypes("/opt/axon/libaxon_pjrt.so")
            except Exception:
                _h._HOOK = None
        return _h._HOOK

    _h.get_axon_ntff_profile_hook = _get_hook
    _h.set_axon_ntff_profile_hook = lambda hook: setattr(_h, "_HOOK", hook)
    sys.modules["antenv.axon_hooks"] = _h

import concourse.bass as bass
import concourse.tile as tile
from concourse import bacc, mybir
from concourse.bass_utils import run_bass_kernel_spmd
from concourse.masks import make_identity

F32 = mybir.dt.float32
BF16 = mybir.dt.bfloat16
AF = mybir.ActivationFunctionType
OP = mybir.AluOpType

P = 128          # partitions
K = 1024         # sequence length
M = 1024         # embedding dim
H = 8            # heads
DH = 128         # head dim
HD = H * DH      # 1024
FF = 4096
KT = K // P      # 8 seq tiles
MT = M // P      # 8 emb tiles
HT = HD // P     # 8 hd tiles
FT = FF // P     # 32 ff tiles
QW = 512         # query-half width
EPS = 1e-10
ISQ = 1.0 / float(np.sqrt(DH))

N_CORES = 8


def _bcast_row_ap(t: bass.AP, width: int) -> bass.AP:
    """DRAM vector -> AP broadcasting one row across 128 partitions."""
    return bass.AP(tensor=t.tensor, offset=t.offset, ap=[[0, P], [1, width]])


def _weave(primary, others):
    """Merge `others` evenly between elements of `primary`."""
    out = []
    n_p, n_o = len(primary), len(others)
    if n_p == 0:
        return list(others)
    oi = 0
    for i, p in enumerate(primary):
        out.append(p)
        want = ((i + 1) * n_o) // n_p
        while oi < want:
            out.append(others[oi])
            oi += 1
    return out


def build_kernel(flags: dict):
    nc = bacc.Bacc("TRN2", target_bir_lowering=False, debug=False,
                   num_devices=N_CORES)
    dram = {}

    def din(name, shape, dt=BF16):
        dram[name] = nc.dram_tensor(name, shape, dt, kind="ExternalInput").ap()

    din("xt_dec", (P, MT * K))
    din("xt_enc", (P, MT * K))
    din("wq_sa", (H, P, MT * DH)); din("wk_sa", (H, P, MT * DH))
    din("wq_ca", (H, P, MT * DH)); din("wk_ca", (H, P, MT * DH))
    din("wv_sa", (MT, P, HD)); din("wv_ca", (MT, P, HD))
    din("wo_sa", (HT, P, M)); din("wo_ca", (HT, P, M))
    din("w1", (FT, P, MT * P)); din("w2", (FT, P, M))
    din("ones", (P,))
    din("dec_nat", (K, M), F32)
    for nm in ("bq_sa", "bk_sa", "bq_ca", "bk_ca"):
        if flags[nm]:
            din(nm, (DH, H), F32)
    for nm in ("bv_sa", "bv_ca", "bo_sa", "bo_ca", "bf2",
               "g1", "b1", "g2", "b2", "g3", "b3"):
        if flags[nm]:
            din(nm, (M,), F32)
    if flags["bf1"]:
        din("bf1", (P, FT), F32)
    out = nc.dram_tensor("out", (K, M), F32, kind="ExternalOutput").ap()

    with tile.TileContext(nc) as tc:
        _emit(nc, tc, dram, out, flags)
    nc.compile()
    return nc


def _emit(nc, tc, dram, out, flags):
    from contextlib import ExitStack

    with ExitStack() as ctx:
        # ---------- persistent pools ----------
        const = ctx.enter_context(tc.tile_pool(name="const", bufs=1))
        wt = ctx.enter_context(tc.tile_pool(name="wt", bufs=3))
        natp = ctx.enter_context(tc.tile_pool(name="natp", bufs=2))
        residp = ctx.enter_context(tc.tile_pool(name="residp", bufs=2))
        statp = ctx.enter_context(tc.tile_pool(name="statp", bufs=4))
        xpool = ctx.enter_context(tc.tile_pool(name="xpool", bufs=1))
        dscr = ctx.enter_context(tc.tile_pool(name="dscr", bufs=1,
                                              space="DRAM"))

        ones_t = const.tile([P, P], BF16, name="ones_t")
        nc.sync.dma_start(out=ones_t, in_=_bcast_row_ap(dram["ones"], P))
        ident = const.tile([P, P], BF16, name="ident")
        make_identity(nc, ident)
        eps_t = const.tile([P, 1], F32, name="eps_t")
        nc.vector.memset(eps_t, EPS)

        bias_tiles = {}
        for nm in ("bq_sa", "bk_sa", "bq_ca", "bk_ca"):
            if flags[nm]:
                t = const.tile([P, H], F32, name=nm + "_t")
                nc.sync.dma_start(out=t, in_=dram[nm])
                bias_tiles[nm] = t
        if flags["bf1"]:
            t = const.tile([P, FT], F32, name="bf1_t")
            nc.sync.dma_start(out=t, in_=dram["bf1"])
            bias_tiles["bf1"] = t
        for nm in ("bv_sa", "bv_ca", "bo_sa", "bo_ca", "bf2",
                   "g1", "b1", "g2", "b2", "g3", "b3"):
            if flags[nm]:
                t = const.tile([P, M], F32, name=nm + "_t")
                nc.sync.dma_start(out=t, in_=_bcast_row_ap(dram[nm], M))
                bias_tiles[nm] = t

        x1_store = dscr.tile([K, M], BF16, name="x1_store")
        x2_store = dscr.tile([K, M], BF16, name="x2_store")

        def new_xt(name):
            return xpool.tile([P, MT, K], BF16, name=name, tag="xt_slot")

        xt = new_xt("decT")
        nc.sync.dma_start(out=xt, in_=dram["xt_dec"].rearrange(
            "p (mt k) -> p mt k", mt=MT))

        # scoped pools that outlive a single phase
        kh_ctx = tc.tile_pool(name="khp", bufs=8)
        khp = kh_ctx.__enter__()
        vcat_ctx = tc.tile_pool(name="vcatp", bufs=1)
        vcatp = vcat_ctx.__enter__()
        wo_ctx = tc.tile_pool(name="wop", bufs=8)
        wop = wo_ctx.__enter__()
        enc_ctx = tc.tile_pool(name="encp", bufs=1)
        encp = enc_ctx.__enter__()
        enc_xt = encp.tile([P, MT, K], BF16, name="encT", tag="enct")
        nc.sync.dma_start(out=enc_xt, in_=dram["xt_enc"].rearrange(
            "p (mt k) -> p mt k", mt=MT))

        # ================= building blocks =================

        def kh_proj_cbs(src_xt, w_name, b_name, h, pspool, dst_holder):
            """Callbacks computing one head's K^T (or Q^T) projection into a
            fresh [P, K] bf16 tile (stored in dst_holder[h])."""
            st = {}

            def dma():
                w = wt.tile([P, MT, DH], BF16, name=f"{w_name}{h}", tag="wt",
                            bufs=3)
                nc.sync.dma_start(out=w, in_=dram[w_name][h].rearrange(
                    "p (mt d) -> p mt d", mt=MT))
                st["w"] = w
                st["q"] = khp.tile([P, K], BF16, name=f"{w_name}h{h}",
                                   tag="khq", bufs=10)
                dst_holder[h] = st["q"]

            def chain(half):
                def cb():
                    pq = pspool.tile([P, QW], F32, name=f"pq_{w_name}{h}{half}",
                                     tag="pq", bufs=2)
                    for mt in range(MT):
                        nc.tensor.matmul(
                            pq, st["w"][:, mt, :],
                            src_xt[:, mt, half * QW:(half + 1) * QW],
                            start=(mt == 0), stop=(mt == MT - 1))
                    d = st["q"][:, half * QW:(half + 1) * QW]
                    if b_name is not None and flags[b_name]:
                        nc.scalar.activation(d, pq, AF.Identity,
                                             bias=bias_tiles[b_name][:, h:h + 1])
                    else:
                        nc.vector.tensor_copy(d, pq)
                return cb

            return [dma, chain(0), chain(1)]

        def v_proj_cbs(src_xt, wv_name, bv_name, vcat, pspool):
            """Callbacks computing vcat [P, KT, HD] bf16 = V^T projection."""
            st = {}

            def dma(mt):
                def cb():
                    w = wt.tile([P, HD], BF16, name=f"{wv_name}{mt}",
                                tag="wv", bufs=8)
                    nc.sync.dma_start(out=w, in_=dram[wv_name][mt])
                    st[mt] = w
                return cb

            def chunk(g, kt):
                def cb():
                    pv = pspool.tile([P, QW], F32, name=f"pv{g}_{kt}",
                                     tag="pv", bufs=2)
                    for mt in range(MT):
                        nc.tensor.matmul(
                            pv, src_xt[:, mt, kt * P:(kt + 1) * P],
                            st[mt][:, g * QW:(g + 1) * QW],
                            start=(mt == 0), stop=(mt == MT - 1))
                    dst = vcat[:, kt, g * QW:(g + 1) * QW]
                    if flags[bv_name]:
                        nc.vector.scalar_tensor_tensor(
                            out=dst, in0=pv, scalar=1.0,
                            in1=bias_tiles[bv_name][:, g * QW:(g + 1) * QW],
                            op0=OP.bypass, op1=OP.add)
                    else:
                        nc.vector.tensor_copy(dst, pv)
                return cb

            cbs = [dma(mt) for mt in range(MT)]
            for g in range(2):
                for kt in range(KT):
                    cbs.append(chunk(g, kt))
            return cbs

        def attention(src_xt, kh_list, vcat, ycat, wq_name, bq_name, pspool,
                      unit_fillers):
            """Pipelined attention units. kh_list/vcat must be ready.
            Emits q projections for heads 1..7 woven between units."""
            fillq = list(unit_fillers)
            with tc.tile_pool(name="attp", bufs=1) as attp:
                qh = {}
                state = {}

                def qproj(h):
                    return kh_proj_cbs(src_xt, wq_name, bq_name, h, pspool,
                                       qh)

                def unit_cbs(h, q):
                    uid = f"{wq_name}{h}_{q}"

                    def sc(kt):
                        def cb():
                            if kt == 0:
                                state[(h, q)] = attp.tile(
                                    [P, KT, QW], BF16, name=f"ex_{uid}",
                                    tag="ex", bufs=2)
                            pss = pspool.tile([P, QW], F32,
                                              name=f"ss_{uid}_{kt}",
                                              tag="pss", bufs=4)
                            nc.tensor.matmul(
                                pss, kh_list[h][:, kt * P:(kt + 1) * P],
                                qh[h][:, q * QW:(q + 1) * QW],
                                start=True, stop=True)
                            nc.scalar.activation(state[(h, q)][:, kt, :], pss,
                                                 AF.Exp, scale=ISQ)
                        return cb

                    def dn(kt):
                        def cb():
                            if kt == 0:
                                state[("d", h, q)] = pspool.tile(
                                    [P, QW], F32, name=f"sd_{uid}",
                                    tag="psd", bufs=1)
                            nc.tensor.matmul(
                                state[("d", h, q)], ones_t,
                                state[(h, q)][:, kt, :],
                                start=(kt == 0), stop=(kt == KT - 1),
                                skip_group_check=True)
                        return cb

                    def rc():
                        r = attp.tile([P, QW], F32, name=f"rc_{uid}",
                                      tag="rc", bufs=2)
                        state[("r", h, q)] = r
                        nc.vector.reciprocal_approx_fast(
                            out=r, in_=state[("d", h, q)])

                    def py(kt):
                        def cb():
                            if kt == 0:
                                state[("y", h, q)] = pspool.tile(
                                    [P, QW], F32, name=f"sy_{uid}",
                                    tag="psy", bufs=1)
                            nc.tensor.matmul(
                                state[("y", h, q)],
                                vcat[:, kt, h * DH:(h + 1) * DH],
                                state[(h, q)][:, kt, :],
                                start=(kt == 0), stop=(kt == KT - 1),
                                skip_group_check=True)
                        return cb

                    def mul():
                        nc.vector.tensor_mul(
                            ycat[:, h, q * QW:(q + 1) * QW],
                            state[("y", h, q)], state[("r", h, q)])

                    scores = [sc(kt) for kt in range(KT)]
                    tail = ([dn(kt) for kt in range(KT)] + [rc]
                            + [py(kt) for kt in range(KT)] + [mul])
                    return scores, tail

                for cb in qproj(0):
                    cb()
                prev_tail = []
                for u in range(2 * H):
                    h, q = u // 2, u % 2
                    scores, tail = unit_cbs(h, q)
                    others = list(prev_tail)
                    if q == 1 and h + 1 < H:
                        others += qproj(h + 1)
                    if fillq:
                        others.append(fillq.pop(0))
                    for cb in _weave(scores, others):
                        cb()
                    prev_tail = tail
                for cb in prev_tail:
                    cb()
                for cb in fillq:
                    cb()

        def ln_tail(z, kt, g_name, b_name, store_dram, to_out, xpool_, pfx="",
                    stats=None, have_sg=0):
            """x = LN(z) (+g/b); DMA to scratch (bf16) or output (f32).
            The normalize-apply runs on the Scalar engine (idle in LN phases):
            x = inv*z + (-mean*inv)."""
            if stats is None:
                stats = statp.tile([P, 2, 6], F32, name=f"st{pfx}{kt}",
                                   tag="stats")
            for sg in range(have_sg, 2):
                nc.vector.bn_stats(out=stats[:, sg, :],
                                   in_=z[:, sg * 512:(sg + 1) * 512])
            mv = statp.tile([P, 2], F32, name=f"mv{pfx}{kt}", tag="mv")
            nc.vector.bn_aggr(out=mv, in_=stats)
            std = statp.tile([P, 1], F32, name=f"sd{pfx}{kt}", tag="std")
            nc.scalar.activation(std, mv[:, 1:2], AF.Sqrt, bias=eps_t)
            inv = statp.tile([P, 1], F32, name=f"iv{pfx}{kt}", tag="inv")
            nc.vector.reciprocal(inv, std)
            nmi = statp.tile([P, 1], F32, name=f"nm{pfx}{kt}", tag="nmi")
            nc.vector.tensor_scalar(out=nmi, in0=mv[:, 0:1], scalar1=inv,
                                    scalar2=-1.0, op0=OP.mult, op1=OP.mult)
            dt = F32 if to_out else BF16
            tag = "xc" if to_out else "xn"
            x = xpool_.tile([P, M], dt, name=f"x{pfx}{kt}", tag=tag, bufs=2)
            nc.scalar.activation(x, z, AF.Identity, bias=nmi, scale=inv)
            if flags[g_name]:
                nc.vector.tensor_mul(x, x, bias_tiles[g_name])
            if flags[b_name]:
                nc.vector.tensor_add(x, x, bias_tiles[b_name])
            if to_out:
                nc.sync.dma_start(out=out[kt * P:(kt + 1) * P, :], in_=x)
            else:
                nc.sync.dma_start(out=store_dram[kt * P:(kt + 1) * P, :],
                                  in_=x)
            return x

        def transpose_into(x, kt, xt_new, pspool):
            for mt in range(MT):
                pt = pspool.tile([P, P], BF16, name=f"ptr{kt}_{mt}", tag="pt",
                                 bufs=2)
                nc.tensor.transpose(pt, x[:, mt * P:(mt + 1) * P], ident)
                nc.vector.tensor_copy(xt_new[:, mt, kt * P:(kt + 1) * P], pt)

        def wo_ln_block(ycat, wots, bo_name, resid_dram, resid_dt, g_name,
                        b_name, store_dram, xt_new, pspool, fillers, pfx):
            """a = ycat @ Wo^T (+bo); z = resid + a; LN tail per kt.
            fillers: callbacks (next stage's projections) woven per kt."""
            fillq = list(fillers)
            per_kt = (len(fillq) + KT - 1) // KT if fillq else 0
            x_prev = None
            for kt in range(KT):
                resid = residp.tile([P, M], resid_dt, name=f"rs{pfx}{kt}",
                                    tag="resid" + pfx)
                nc.sync.dma_start(
                    out=resid, in_=resid_dram[kt * P:(kt + 1) * P, :])
                z = natp.tile([P, M], F32, name=f"z{pfx}{kt}", tag="z",
                              bufs=4)
                for mh in range(2):
                    pa = pspool.tile([P, QW], F32, name=f"pa{pfx}{kt}_{mh}",
                                     tag="pa", bufs=2)
                    for ht in range(HT):
                        nc.tensor.matmul(
                            pa, ycat[:, ht, kt * P:(kt + 1) * P],
                            wots[ht][:, mh * QW:(mh + 1) * QW],
                            start=(ht == 0), stop=(ht == HT - 1))
                    sl = slice(mh * QW, (mh + 1) * QW)
                    if flags[bo_name]:
                        nc.vector.scalar_tensor_tensor(
                            out=z[:, sl], in0=pa, scalar=1.0,
                            in1=bias_tiles[bo_name][:, sl],
                            op0=OP.bypass, op1=OP.add)
                        nc.vector.tensor_add(z[:, sl], z[:, sl],
                                             resid[:, sl])
                    else:
                        nc.vector.tensor_add(z[:, sl], pa, resid[:, sl])
                    for _ in range(per_kt // 2):
                        if fillq:
                            fillq.pop(0)()
                x = ln_tail(z, kt, g_name, b_name, store_dram, False, natp,
                            pfx)
                # transposes lag one kt so the PE never waits on the
                # DVE LayerNorm chain of the current kt
                if x_prev is not None:
                    transpose_into(x_prev, kt - 1, xt_new, pspool)
                x_prev = x
            transpose_into(x_prev, KT - 1, xt_new, pspool)
            for cb in fillq:
                cb()

        def load_wo(wo_name):
            """DMA-issue callbacks for the 8 Wo tiles (no PE cost)."""
            tiles = []
            cbs = []
            for ht in range(HT):
                def cb(ht=ht):
                    w = wop.tile([P, M], BF16, name=f"{wo_name}{ht}",
                                 tag="wo8")
                    nc.sync.dma_start(out=w, in_=dram[wo_name][ht])
                    tiles.append(w)
                cbs.append(cb)
            return tiles, cbs

        # ================= S1: self-attention setup =================
        kh_sa = {}
        vcat_sa = vcatp.tile([P, KT, HD], BF16, name="vcat_sa", tag="vcat")
        with nc.named_scope("s1_setup"), \
                tc.tile_pool(name="ps_s1", bufs=1, space="PSUM") as ps1:
            kcbs = []
            for h in range(H):
                kcbs += kh_proj_cbs(xt, "wk_sa", "bk_sa", h, ps1, kh_sa)
            vcbs = v_proj_cbs(xt, "wv_sa", "bv_sa", vcat_sa, ps1)
            # dmas first (no PE), then weave chains
            dmas = vcbs[:MT]
            for cb in dmas:
                cb()
            for cb in _weave(kcbs, vcbs[MT:]):
                cb()

        # ================= S2: self-attention units =================
        wots_sa, wo_sa_cbs = load_wo("wo_sa")
        with tc.tile_pool(name="ya_sa", bufs=1) as yap:
            ycat = yap.tile([P, H, K], BF16, name="ycat_sa", tag="ycat")
            with nc.named_scope("s2_sa_att"), \
                    tc.tile_pool(name="ps_s2", bufs=1, space="PSUM") as ps2:
                attention(xt, kh_sa, vcat_sa, ycat, "wq_sa", "bq_sa", ps2,
                          wo_sa_cbs)

            # ============= S3: sa Wo+LN, fillers = ca K/V proj =============
            x1t = new_xt("x1T")
            kh_ca = {}
            vcat_ca = vcatp.tile([P, KT, HD], BF16, name="vcat_ca",
                                 tag="vcat")
            with nc.named_scope("s3_sa_wo_ln"), \
                    tc.tile_pool(name="ps_s3", bufs=1, space="PSUM") as ps3:
                fillers = []
                kcbs = []
                for h in range(H):
                    kcbs += kh_proj_cbs(enc_xt, "wk_ca", "bk_ca", h, ps3,
                                        kh_ca)
                vcbs = v_proj_cbs(enc_xt, "wv_ca", "bv_ca", vcat_ca, ps3)
                fillers = vcbs[:MT] + _weave(kcbs, vcbs[MT:])
                wo_ln_block(ycat, wots_sa, "bo_sa", dram["dec_nat"], F32,
                            "g1", "b1", x1_store, x1t, ps3, fillers, "a")
        xt = x1t

        # ================= S4: cross-attention units =================
        wots_ca, wo_ca_cbs = load_wo("wo_ca")
        with tc.tile_pool(name="ya_ca", bufs=1) as yap:
            ycat = yap.tile([P, H, K], BF16, name="ycat_ca", tag="ycat")
            with nc.named_scope("s4_ca_att"), \
                    tc.tile_pool(name="ps_s4", bufs=1, space="PSUM") as ps4:
                attention(xt, kh_ca, vcat_ca, ycat, "wq_ca", "bq_ca", ps4,
                          wo_ca_cbs)

            # ============= S5: ca Wo+LN =============
            x2t = new_xt("x2T")
            x2r_tiles = []

            def x2r_dma(kt):
                def cb():
                    r = residp.tile([P, M], BF16, name=f"x2r{kt}", tag="x2r",
                                    bufs=8)
                    nc.sync.dma_start(
                        out=r, in_=x2_store[kt * P:(kt + 1) * P, :])
                    x2r_tiles.append(r)
                return cb

            with nc.named_scope("s5_ca_wo_ln"), \
                    tc.tile_pool(name="ps_s5", bufs=1, space="PSUM") as ps5:
                wo_ln_block(ycat, wots_ca, "bo_ca", x1_store, BF16,
                            "g2", "b2", x2_store, x2t, ps5, [], "b")
        xt = x2t
        enc_ctx.__exit__(None, None, None)
        wo_ctx.__exit__(None, None, None)
        vcat_ctx.__exit__(None, None, None)
        kh_ctx.__exit__(None, None, None)

        # ================= S6: feed-forward =================
        with tc.tile_pool(name="rtp", bufs=1) as rtp, \
                nc.named_scope("s6_ffn"):
            rt = rtp.tile([P, FT, K], BF16, name="rt", tag="rt")
            # FFN1: ft-outer, both query halves per ft; W1 loaded once.
            with tc.tile_pool(name="ps_f1", bufs=1, space="PSUM") as psf:
                for kt in range(KT):
                    x2r_dma(kt)()
                for ft in range(FT):
                    w1t = wt.tile([P, MT, P], BF16, name=f"w1_{ft}", tag="wt")
                    nc.sync.dma_start(out=w1t, in_=dram["w1"][ft].rearrange(
                        "p (mt d) -> p mt d", mt=MT))
                    for kqh in range(2):
                        pf = psf.tile([P, QW], F32, name=f"pf{ft}_{kqh}",
                                      tag="pf", bufs=3)
                        for mt in range(MT):
                            nc.tensor.matmul(
                                pf, w1t[:, mt, :],
                                xt[:, mt, kqh * QW:(kqh + 1) * QW],
                                start=(mt == 0), stop=(mt == MT - 1))
                        dst = rt[:, ft, kqh * QW:(kqh + 1) * QW]
                        if flags["bf1"]:
                            nc.scalar.activation(
                                dst, pf, AF.Relu,
                                bias=bias_tiles["bf1"][:, ft:ft + 1])
                        else:
                            nc.scalar.activation(dst, pf, AF.Relu)
            # FFN2: kt-groups of 4, W2 streamed per (group, mh); stats of the
            # first z-half computed during the second half's matmuls, LN of
            # group 0 overlaps group 1's matmuls.
            with tc.tile_pool(name="ps_f2", bufs=1, space="PSUM") as psf2, \
                    tc.tile_pool(name="w2s", bufs=6) as w2sp:
                for ktg in range(2):
                    z3s, stats_s, paccs = {}, {}, {}
                    for mh in range(2):
                        for ft in range(FT):
                            w2t = w2sp.tile([P, QW], BF16,
                                            name=f"w2_{ktg}_{mh}_{ft}",
                                            tag="w2s", bufs=6)
                            nc.sync.dma_start(
                                out=w2t,
                                in_=dram["w2"][ft, :, mh * QW:(mh + 1) * QW])
                            for ks in range(4):
                                kt = ktg * 4 + ks
                                if ft == 0:
                                    paccs[ks] = psf2.tile(
                                        [P, QW], F32, name=f"po{kt}_{mh}",
                                        tag="pacc", bufs=8)
                                nc.tensor.matmul(
                                    paccs[ks],
                                    rt[:, ft, kt * P:(kt + 1) * P], w2t,
                                    start=(ft == 0), stop=(ft == FT - 1))
                        sl = slice(mh * QW, (mh + 1) * QW)
                        for ks in range(4):
                            kt = ktg * 4 + ks
                            if mh == 0:
                                z3s[ks] = natp.tile([P, M], F32,
                                                    name=f"z3_{kt}",
                                                    tag="z", bufs=4)
                                stats_s[ks] = statp.tile(
                                    [P, 2, 6], F32, name=f"stc{kt}",
                                    tag="stats")
                            z3 = z3s[ks]
                            if flags["bf2"]:
                                nc.vector.scalar_tensor_tensor(
                                    out=z3[:, sl], in0=paccs[ks], scalar=1.0,
                                    in1=bias_tiles["bf2"][:, sl],
                                    op0=OP.bypass, op1=OP.add)
                                nc.vector.tensor_add(z3[:, sl], z3[:, sl],
                                                     x2r_tiles[kt][:, sl])
                            else:
                                nc.vector.tensor_add(z3[:, sl], paccs[ks],
                                                     x2r_tiles[kt][:, sl])
                            if mh == 0:
                                nc.vector.bn_stats(out=stats_s[ks][:, 0, :],
                                                   in_=z3[:, sl])
                    for ks in range(4):
                        kt = ktg * 4 + ks
                        ln_tail(z3s[ks], kt, "g3", "b3", None, True, natp,
                                "c", stats=stats_s[ks], have_sg=1)


def _pack_inputs(inputs: dict):
    """Host-side packing -> (flags, per-core in_maps)."""
    f32 = np.float32
    bf = ml_dtypes.bfloat16
    dec = np.asarray(inputs["dec"], f32)
    enc = np.asarray(inputs["enc"], f32)

    def nz(x):
        return bool(np.any(np.asarray(x) != 0.0))

    flags = {
        "bq_sa": nz(inputs["bq_sa"]), "bk_sa": nz(inputs["bk_sa"]),
        "bv_sa": nz(inputs["bv_sa"]), "bo_sa": nz(inputs["bo_sa"]),
        "bq_ca": nz(inputs["bq_ca"]), "bk_ca": nz(inputs["bk_ca"]),
        "bv_ca": nz(inputs["bv_ca"]), "bo_ca": nz(inputs["bo_ca"]),
        "bf1": nz(inputs["bf1"]), "bf2": nz(inputs["bf2"]),
        "g1": bool(np.any(np.asarray(inputs["g1"]) != 1.0)),
        "b1": nz(inputs["b1"]),
        "g2": bool(np.any(np.asarray(inputs["g2"]) != 1.0)),
        "b2": nz(inputs["b2"]),
        "g3": bool(np.any(np.asarray(inputs["g3"]) != 1.0)),
        "b3": nz(inputs["b3"]),
    }

    def qk_pack(w):
        w = np.asarray(w, f32)  # (H, DH, M)
        return (w.transpose(0, 2, 1).reshape(H, MT, P, DH)
                .transpose(0, 2, 1, 3).reshape(H, P, MT * DH)).astype(bf)

    def v_pack(w):
        w = np.asarray(w, f32)  # (H, DH, M) -> WvT [m, hd]
        wt_ = w.transpose(2, 0, 1).reshape(M, HD)
        return wt_.reshape(MT, P, HD).astype(bf)

    def o_pack(w):  # (M, HD) -> WoT (HD, M) -> (HT, P, M)
        return (np.ascontiguousarray(np.asarray(w, f32).T)
                .reshape(HT, P, M).astype(bf))

    W1 = np.asarray(inputs["W1"], f32)
    W2 = np.asarray(inputs["W2"], f32)
    shared = {
        "wq_sa": qk_pack(inputs["Wq_sa"]), "wk_sa": qk_pack(inputs["Wk_sa"]),
        "wv_sa": v_pack(inputs["Wv_sa"]), "wo_sa": o_pack(inputs["Wo_sa"]),
        "wq_ca": qk_pack(inputs["Wq_ca"]), "wk_ca": qk_pack(inputs["Wk_ca"]),
        "wv_ca": v_pack(inputs["Wv_ca"]), "wo_ca": o_pack(inputs["Wo_ca"]),
        "w1": (W1.reshape(FT, P, MT, P).transpose(0, 3, 2, 1)
               .reshape(FT, P, MT * P)).astype(bf),
        "w2": np.ascontiguousarray(W2.T).reshape(FT, P, M).astype(bf),
        "ones": np.ones(P, bf),
    }
    for nm in ("bq_sa", "bk_sa", "bq_ca", "bk_ca"):
        if flags[nm]:
            shared[nm] = np.ascontiguousarray(np.asarray(inputs[nm], f32).T)
    for nm in ("bv_sa", "bv_ca"):
        if flags[nm]:
            shared[nm] = np.asarray(inputs[nm], f32).reshape(HD)
    for nm in ("bo_sa", "bo_ca", "bf2", "g1", "b1", "g2", "b2", "g3", "b3"):
        if flags[nm]:
            shared[nm] = np.asarray(inputs[nm], f32)
    if flags["bf1"]:
        shared["bf1"] = np.ascontiguousarray(
            np.asarray(inputs["bf1"], f32).reshape(FT, P).T)

    def xt_pack(x):  # (K, M) -> transposed, partition-contiguous (P, MT*K)
        return (x.T.reshape(MT, P, K).transpose(1, 0, 2)
                .reshape(P, MT * K)).astype(bf)

    in_maps = []
    for c in range(N_CORES):
        m = dict(shared)
        m["xt_dec"] = xt_pack(dec[c])
        m["xt_enc"] = xt_pack(enc[c])
        m["dec_nat"] = np.ascontiguousarray(dec[c])
        in_maps.append(m)
    return flags, in_maps


_NC_CACHE: dict = {}


def kernel(**inputs) -> np.ndarray:
    flags, in_maps = _pack_inputs(inputs)
    key = tuple(sorted(flags.items()))
    if key not in _NC_CACHE:
        _NC_CACHE[key] = build_kernel(flags)
    nc = _NC_CACHE[key]
    res = run_bass_kernel_spmd(nc, in_maps, core_ids=list(range(N_CORES)))
    return np.stack([res.results[c]["out"] for c in range(N_CORES)])


# revision 23
# speedup vs baseline: 1.0295x; 1.0295x over previous
"""Trainium2 Bass kernel for nn_DecoderBlock (dense transformer decoder block).

Sharding: data-parallel over batch N=8 -> one batch element per NeuronCore.
Zero collectives; weights replicated to every core.

Per-core computation (K=1024 tokens, M=1024 emb, H=8 heads, DH=128, FF=4096):
  a1 = MHA(dec, dec);  x1 = LN(dec + a1)
  a2 = MHA(x1, enc);   x2 = LN(x1 + a2)
  ff = relu(x2 @ W1.T) @ W2.T;  out = LN(x2 + ff)

v2 design notes:
  - All matmul operands are bf16 (same 1 cyc/row PE rate as fp32r, half the
    DMA/SBUF). Accumulation, LN, residual math stay fp32. Measured numpy
    rel-err for this scheme: 5.2e-3.
  - Attention is software-pipelined at (head, query-half) "unit" granularity
    with a one-unit lag: scores of unit u interleave with denominator/PV
    matmuls of unit u-1, so the PE never waits on the Scalar-engine exp.
  - Softmax denominator via ones-matmul broadcast; reciprocal via the fast
    custom DVE op; normalization folded into the ycat multiply.
  - K projections + V projections of the *next* attention are emitted as
    fillers inside the current Wo+LN block, so those phases stay PE-dense.
  - FFN: ft-outer FFN1 (W1 loaded once), kt-outer FFN2 (W2 resident in SBUF,
    LN of token tile kt overlaps matmuls of kt+1).
"""
import sys

sys.path.insert(0, "/opt/trn_rl_repo")

import numpy as np
import ml_dtypes

# antenv.axon_hooks shim (needed only if BASS_TRACE is set; the agent image's
# read-only antenv package lacks this module).
try:
    from antenv import axon_hooks as _ah  # noqa: F401
except ImportError:
    import types as _types

    _h = _types.ModuleType("antenv.axon_hooks")
    _h._HOOK = None

    def _get_hook():
        if _h._HOOK is None:
            try:
                from trn_agent_boot.trn_boot import _ntff_profile_via_ctypes

                _h._HOOK = _ntff_profile_via_ctypes("/opt/axon/libaxon_pjrt.so")
            except Exception:
                _h._HOOK = None
        return _h._HOOK

    _h.get_axon_ntff_profile_hook = _get_hook
    _h.set_axon_ntff_profile_hook = lambda hook: setattr(_h, "_HOOK", hook)
    sys.modules["antenv.axon_hooks"] = _h

import concourse.bass as bass
import concourse.tile as tile
from concourse import bacc, mybir
from concourse.bass_utils import run_bass_kernel_spmd
from concourse.masks import make_identity

F32 = mybir.dt.float32
BF16 = mybir.dt.bfloat16
AF = mybir.ActivationFunctionType
OP = mybir.AluOpType

P = 128          # partitions
K = 1024         # sequence length
M = 1024         # embedding dim
H = 8            # heads
DH = 128         # head dim
HD = H * DH      # 1024
FF = 4096
KT = K // P      # 8 seq tiles
MT = M // P      # 8 emb tiles
HT = HD // P     # 8 hd tiles
FT = FF // P     # 32 ff tiles
QW = 512         # query-half width
EPS = 1e-10
ISQ = 1.0 / float(np.sqrt(DH))

N_CORES = 8


def _bcast_row_ap(t: bass.AP, width: int) -> bass.AP:
    """DRAM vector -> AP broadcasting one row across 128 partitions."""
    return bass.AP(tensor=t.tensor, offset=t.offset, ap=[[0, P], [1, width]])


def _weave(primary, others):
    """Merge `others` evenly between elements of `primary`."""
    out = []
    n_p, n_o = len(primary), len(others)
    if n_p == 0:
        return list(others)
    oi = 0
    for i, p in enumerate(primary):
        out.append(p)
        want = ((i + 1) * n_o) // n_p
        while oi < want:
            out.append(others[oi])
            oi += 1
    return out


def build_kernel(flags: dict):
    nc = bacc.Bacc("TRN2", target_bir_lowering=False, debug=False,
                   num_devices=N_CORES)
    dram = {}

    def din(name, shape, dt=BF16):
        dram[name] = nc.dram_tensor(name, shape, dt, kind="ExternalInput").ap()

    din("xt_dec", (P, MT * K))
    din("xt_enc", (P, MT * K))
    din("wq_sa", (H, P, MT * DH)); din("wk_sa", (H, P, MT * DH))
    din("wq_ca", (H, P, MT * DH)); din("wk_ca", (H, P, MT * DH))
    din("wv_sa", (MT, P, HD)); din("wv_ca", (MT, P, HD))
    din("wo_sa", (HT, P, M)); din("wo_ca", (HT, P, M))
    din("w1", (FT, P, MT * P)); din("w2", (FT, P, M))
    din("ones", (P,))
    din("dec_nat", (K, M), F32)
    for nm in ("bq_sa", "bk_sa", "bq_ca", "bk_ca"):
        if flags[nm]:
            din(nm, (DH, H), F32)
    for nm in ("bv_sa", "bv_ca", "bo_sa", "bo_ca", "bf2",
               "g1", "b1", "g2", "b2", "g3", "b3"):
        if flags[nm]:
            din(nm, (M,), F32)
    if flags["bf1"]:
        din("bf1", (P, FT), F32)
    out = nc.dram_tensor("out", (K, M), F32, kind="ExternalOutput").ap()

    with tile.TileContext(nc) as tc:
        _emit(nc, tc, dram, out, flags)
    nc.compile()
    return nc


def _emit(nc, tc, dram, out, flags):
    from contextlib import ExitStack

    with ExitStack() as ctx:
        # ---------- persistent pools ----------
        const = ctx.enter_context(tc.tile_pool(name="const", bufs=1))
        wt = ctx.enter_context(tc.tile_pool(name="wt", bufs=3))
        natp = ctx.enter_context(tc.tile_pool(name="natp", bufs=2))
        residp = ctx.enter_context(tc.tile_pool(name="residp", bufs=2))
        statp = ctx.enter_context(tc.tile_pool(name="statp", bufs=4))
        xpool = ctx.enter_context(tc.tile_pool(name="xpool", bufs=1))
        dscr = ctx.enter_context(tc.tile_pool(name="dscr", bufs=1,
                                              space="DRAM"))

        ones_t = const.tile([P, P], BF16, name="ones_t")
        nc.sync.dma_start(out=ones_t, in_=_bcast_row_ap(dram["ones"], P))
        ident = const.tile([P, P], BF16, name="ident")
        make_identity(nc, ident)
        eps_t = const.tile([P, 1], F32, name="eps_t")
        nc.vector.memset(eps_t, EPS)

        bias_tiles = {}
        for nm in ("bq_sa", "bk_sa", "bq_ca", "bk_ca"):
            if flags[nm]:
                t = const.tile([P, H], F32, name=nm + "_t")
                nc.sync.dma_start(out=t, in_=dram[nm])
                bias_tiles[nm] = t
        if flags["bf1"]:
            t = const.tile([P, FT], F32, name="bf1_t")
            nc.sync.dma_start(out=t, in_=dram["bf1"])
            bias_tiles["bf1"] = t
        for nm in ("bv_sa", "bv_ca", "bo_sa", "bo_ca", "bf2",
                   "g1", "b1", "g2", "b2", "g3", "b3"):
            if flags[nm]:
                t = const.tile([P, M], F32, name=nm + "_t")
                nc.sync.dma_start(out=t, in_=_bcast_row_ap(dram[nm], M))
                bias_tiles[nm] = t

        x1_store = dscr.tile([K, M], BF16, name="x1_store")

        def new_xt(name):
            return xpool.tile([P, MT, K], BF16, name=name, tag="xt_slot")

        xt = new_xt("decT")
        nc.sync.dma_start(out=xt, in_=dram["xt_dec"].rearrange(
            "p (mt k) -> p mt k", mt=MT))

        # scoped pools that outlive a single phase
        kh_ctx = tc.tile_pool(name="khp", bufs=8)
        khp = kh_ctx.__enter__()
        vcat_ctx = tc.tile_pool(name="vcatp", bufs=1)
        vcatp = vcat_ctx.__enter__()
        wo_ctx = tc.tile_pool(name="wop", bufs=8)
        wop = wo_ctx.__enter__()
        enc_ctx = tc.tile_pool(name="encp", bufs=1)
        encp = enc_ctx.__enter__()
        enc_xt = encp.tile([P, MT, K], BF16, name="encT", tag="enct")

        def enc_dma():
            # deferred to S2 so startup DMAs (decT, sa weights) go first
            nc.sync.dma_start(out=enc_xt, in_=dram["xt_enc"].rearrange(
                "p (mt k) -> p mt k", mt=MT))

        # ================= building blocks =================

        def kh_proj_cbs(src_xt, w_name, b_name, h, pspool, dst_holder):
            """Callbacks computing one head's K^T (or Q^T) projection into a
            fresh [P, K] bf16 tile (stored in dst_holder[h])."""
            st = {}

            def dma():
                w = wt.tile([P, MT, DH], BF16, name=f"{w_name}{h}", tag="wt",
                            bufs=3)
                nc.sync.dma_start(out=w, in_=dram[w_name][h].rearrange(
                    "p (mt d) -> p mt d", mt=MT))
                st["w"] = w
                st["q"] = khp.tile([P, K], BF16, name=f"{w_name}h{h}",
                                   tag="khq", bufs=10)
                dst_holder[h] = st["q"]

            def chain(half):
                def cb():
                    pq = pspool.tile([P, QW], F32, name=f"pq_{w_name}{h}{half}",
                                     tag="pq", bufs=2)
                    for mt in range(MT):
                        nc.tensor.matmul(
                            pq, st["w"][:, mt, :],
                            src_xt[:, mt, half * QW:(half + 1) * QW],
                            start=(mt == 0), stop=(mt == MT - 1))
                    d = st["q"][:, half * QW:(half + 1) * QW]
                    if b_name is not None and flags[b_name]:
                        nc.scalar.activation(d, pq, AF.Identity,
                                             bias=bias_tiles[b_name][:, h:h + 1])
                    else:
                        nc.vector.tensor_copy(d, pq)
                return cb

            return [dma, chain(0), chain(1)]

        def v_proj_cbs(src_xt, wv_name, bv_name, vcat, pspool):
            """Callbacks computing vcat [P, KT, HD] bf16 = V^T projection."""
            st = {}

            def dma(mt):
                def cb():
                    w = wt.tile([P, HD], BF16, name=f"{wv_name}{mt}",
                                tag="wv", bufs=8)
                    nc.sync.dma_start(out=w, in_=dram[wv_name][mt])
                    st[mt] = w
                return cb

            def chunk(g, kt):
                def cb():
                    pv = pspool.tile([P, QW], F32, name=f"pv{g}_{kt}",
                                     tag="pv", bufs=2)
                    for mt in range(MT):
                        nc.tensor.matmul(
                            pv, src_xt[:, mt, kt * P:(kt + 1) * P],
                            st[mt][:, g * QW:(g + 1) * QW],
                            start=(mt == 0), stop=(mt == MT - 1))
                    dst = vcat[:, kt, g * QW:(g + 1) * QW]
                    if flags[bv_name]:
                        nc.vector.scalar_tensor_tensor(
                            out=dst, in0=pv, scalar=1.0,
                            in1=bias_tiles[bv_name][:, g * QW:(g + 1) * QW],
                            op0=OP.bypass, op1=OP.add)
                    else:
                        nc.vector.tensor_copy(dst, pv)
                return cb

            cbs = [dma(mt) for mt in range(MT)]
            for g in range(2):
                for kt in range(KT):
                    cbs.append(chunk(g, kt))
            return cbs

        def attention(src_xt, kh_list, vcat, ycat, wq_name, bq_name, pspool,
                      unit_fillers):
            """Pipelined attention units. kh_list/vcat must be ready.
            Emits q projections for heads 1..7 woven between units."""
            fillq = list(unit_fillers)
            with tc.tile_pool(name="attp", bufs=1) as attp:
                qh = {}
                state = {}

                def qproj(h):
                    return kh_proj_cbs(src_xt, wq_name, bq_name, h, pspool,
                                       qh)

                def unit_cbs(h, q):
                    uid = f"{wq_name}{h}_{q}"

                    def sc(kt):
                        def cb():
                            if kt == 0:
                                state[(h, q)] = attp.tile(
                                    [P, KT, QW], BF16, name=f"ex_{uid}",
                                    tag="ex", bufs=2)
                            pss = pspool.tile([P, QW], F32,
                                              name=f"ss_{uid}_{kt}",
                                              tag="pss", bufs=4)
                            nc.tensor.matmul(
                                pss, kh_list[h][:, kt * P:(kt + 1) * P],
                                qh[h][:, q * QW:(q + 1) * QW],
                                start=True, stop=True)
                            nc.scalar.activation(state[(h, q)][:, kt, :], pss,
                                                 AF.Exp, scale=ISQ)
                        return cb

                    def dn(kt):
                        def cb():
                            if kt == 0:
                                state[("d", h, q)] = pspool.tile(
                                    [P, QW], F32, name=f"sd_{uid}",
                                    tag="psd", bufs=1)
                            nc.tensor.matmul(
                                state[("d", h, q)], ones_t,
                                state[(h, q)][:, kt, :],
                                start=(kt == 0), stop=(kt == KT - 1),
                                skip_group_check=True)
                        return cb

                    def rc():
                        r = attp.tile([P, QW], F32, name=f"rc_{uid}",
                                      tag="rc", bufs=2)
                        state[("r", h, q)] = r
                        nc.vector.reciprocal_approx_fast(
                            out=r, in_=state[("d", h, q)])

                    def py(kt):
                        def cb():
                            if kt == 0:
                                state[("y", h, q)] = pspool.tile(
                                    [P, QW], F32, name=f"sy_{uid}",
                                    tag="psy", bufs=1)
                            nc.tensor.matmul(
                                state[("y", h, q)],
                                vcat[:, kt, h * DH:(h + 1) * DH],
                                state[(h, q)][:, kt, :],
                                start=(kt == 0), stop=(kt == KT - 1),
                                skip_group_check=True)
                        return cb

                    def mul():
                        nc.vector.tensor_mul(
                            ycat[:, h, q * QW:(q + 1) * QW],
                            state[("y", h, q)], state[("r", h, q)])

                    scores = [sc(kt) for kt in range(KT)]
                    tail = ([dn(kt) for kt in range(KT)] + [rc]
                            + [py(kt) for kt in range(KT)] + [mul])
                    return scores, tail

                for cb in qproj(0):
                    cb()
                prev_tail = []
                for u in range(2 * H):
                    h, q = u // 2, u % 2
                    scores, tail = unit_cbs(h, q)
                    others = list(prev_tail)
                    if q == 1 and h + 1 < H:
                        others += qproj(h + 1)
                    if fillq:
                        others.append(fillq.pop(0))
                    for cb in _weave(scores, others):
                        cb()
                    prev_tail = tail
                for cb in prev_tail:
                    cb()
                for cb in fillq:
                    cb()

        def ln_tail(z, kt, g_name, b_name, store_dram, to_out, xpool_, pfx="",
                    stats=None, have_sg=0):
            """x = LN(z) (+g/b); DMA to scratch (bf16) or output (f32).
            The normalize-apply runs on the Scalar engine (idle in LN phases):
            x = inv*z + (-mean*inv)."""
            if stats is None:
                stats = statp.tile([P, 2, 6], F32, name=f"st{pfx}{kt}",
                                   tag="stats")
            for sg in range(have_sg, 2):
                nc.vector.bn_stats(out=stats[:, sg, :],
                                   in_=z[:, sg * 512:(sg + 1) * 512])
            mv = statp.tile([P, 2], F32, name=f"mv{pfx}{kt}", tag="mv")
            nc.vector.bn_aggr(out=mv, in_=stats)
            std = statp.tile([P, 1], F32, name=f"sd{pfx}{kt}", tag="std")
            nc.scalar.activation(std, mv[:, 1:2], AF.Sqrt, bias=eps_t)
            inv = statp.tile([P, 1], F32, name=f"iv{pfx}{kt}", tag="inv")
            nc.vector.reciprocal(inv, std)
            nmi = statp.tile([P, 1], F32, name=f"nm{pfx}{kt}", tag="nmi")
            nc.vector.tensor_scalar(out=nmi, in0=mv[:, 0:1], scalar1=inv,
                                    scalar2=-1.0, op0=OP.mult, op1=OP.mult)
            if to_out:
                dt, tag, bufs = F32, "xc", 2
            elif store_dram is None:
                dt, tag, bufs = BF16, "x2r", 8   # stays resident for FFN
            else:
                dt, tag, bufs = BF16, "xn", 2
            x = xpool_.tile([P, M], dt, name=f"x{pfx}{kt}", tag=tag,
                            bufs=bufs)
            nc.scalar.activation(x, z, AF.Identity, bias=nmi, scale=inv)
            if flags[g_name]:
                nc.vector.tensor_mul(x, x, bias_tiles[g_name])
            if flags[b_name]:
                nc.vector.tensor_add(x, x, bias_tiles[b_name])
            if to_out:
                nc.sync.dma_start(out=out[kt * P:(kt + 1) * P, :], in_=x)
            elif store_dram is not None:
                nc.sync.dma_start(out=store_dram[kt * P:(kt + 1) * P, :],
                                  in_=x)
            return x

        def transpose_into(x, kt, xt_new, pspool):
            for mt in range(MT):
                pt = pspool.tile([P, P], BF16, name=f"ptr{kt}_{mt}", tag="pt",
                                 bufs=2)
                nc.tensor.transpose(pt, x[:, mt * P:(mt + 1) * P], ident)
                nc.vector.tensor_copy(xt_new[:, mt, kt * P:(kt + 1) * P], pt)

        def wo_ln_block(ycat, wots, bo_name, resid_dram, resid_dt, g_name,
                        b_name, store_dram, xt_new, pspool, fillers, pfx,
                        x_out=None):
            """a = ycat @ Wo^T (+bo); z = resid + a; LN tail per kt.
            fillers: callbacks (next stage's projections) woven per kt."""
            fillq = list(fillers)
            per_kt = (len(fillq) + KT - 1) // KT if fillq else 0
            x_prev = None
            for kt in range(KT):
                resid = residp.tile([P, M], resid_dt, name=f"rs{pfx}{kt}",
                                    tag="resid" + pfx)
                nc.sync.dma_start(
                    out=resid, in_=resid_dram[kt * P:(kt + 1) * P, :])
                z = natp.tile([P, M], F32, name=f"z{pfx}{kt}", tag="z",
                              bufs=4)
                for mh in range(2):
                    pa = pspool.tile([P, QW], F32, name=f"pa{pfx}{kt}_{mh}",
                                     tag="pa", bufs=2)
                    for ht in range(HT):
                        nc.tensor.matmul(
                            pa, ycat[:, ht, kt * P:(kt + 1) * P],
                            wots[ht][:, mh * QW:(mh + 1) * QW],
                            start=(ht == 0), stop=(ht == HT - 1))
                    sl = slice(mh * QW, (mh + 1) * QW)
                    if flags[bo_name]:
                        nc.vector.scalar_tensor_tensor(
                            out=z[:, sl], in0=pa, scalar=1.0,
                            in1=bias_tiles[bo_name][:, sl],
                            op0=OP.bypass, op1=OP.add)
                        nc.vector.tensor_add(z[:, sl], z[:, sl],
                                             resid[:, sl])
                    else:
                        nc.vector.tensor_add(z[:, sl], pa, resid[:, sl])
                    for _ in range(per_kt // 2):
                        if fillq:
                            fillq.pop(0)()
                x = ln_tail(z, kt, g_name, b_name, store_dram, False, natp,
                            pfx)
                if x_out is not None:
                    x_out.append(x)
                # transposes lag one kt so the PE never waits on the
                # DVE LayerNorm chain of the current kt
                if x_prev is not None:
                    transpose_into(x_prev, kt - 1, xt_new, pspool)
                x_prev = x
            transpose_into(x_prev, KT - 1, xt_new, pspool)
            for cb in fillq:
                cb()

        def load_wo(wo_name):
            """DMA-issue callbacks for the 8 Wo tiles (no PE cost)."""
            tiles = []
            cbs = []
            for ht in range(HT):
                def cb(ht=ht):
                    w = wop.tile([P, M], BF16, name=f"{wo_name}{ht}",
                                 tag="wo8")
                    nc.sync.dma_start(out=w, in_=dram[wo_name][ht])
                    tiles.append(w)
                cbs.append(cb)
            return tiles, cbs

        # ================= S1: self-attention setup =================
        # DMA order matters for startup: decT already issued; wk0/wk1 next so
        # the first kh chain starts ~6us in, then wv, then the rest.
        kh_sa = {}
        vcat_sa = vcatp.tile([P, KT, HD], BF16, name="vcat_sa", tag="vcat")
        with nc.named_scope("s1_setup"), \
                tc.tile_pool(name="ps_s1", bufs=1, space="PSUM") as ps1:
            kall = [kh_proj_cbs(xt, "wk_sa", "bk_sa", h, ps1, kh_sa)
                    for h in range(H)]
            vcbs = v_proj_cbs(xt, "wv_sa", "bv_sa", vcat_sa, ps1)
            kall[0][0](); kall[1][0]()          # wk0, wk1 DMAs
            for cb in vcbs[:MT]:                # wv DMAs
                cb()
            for h in range(2, H):               # remaining wk DMAs
                kall[h][0]()
            kchains = [cb for h in range(H) for cb in kall[h][1:]]
            for cb in _weave(kchains, vcbs[MT:]):
                cb()

        # ================= S2: self-attention units =================
        wots_sa, wo_sa_cbs = load_wo("wo_sa")
        with tc.tile_pool(name="ya_sa", bufs=1) as yap:
            ycat = yap.tile([P, H, K], BF16, name="ycat_sa", tag="ycat")
            with nc.named_scope("s2_sa_att"), \
                    tc.tile_pool(name="ps_s2", bufs=1, space="PSUM") as ps2:
                attention(xt, kh_sa, vcat_sa, ycat, "wq_sa", "bq_sa", ps2,
                          [enc_dma] + wo_sa_cbs)

            # ============= S3: sa Wo+LN, fillers = ca K/V proj =============
            x1t = new_xt("x1T")
            kh_ca = {}
            vcat_ca = vcatp.tile([P, KT, HD], BF16, name="vcat_ca",
                                 tag="vcat")
            with nc.named_scope("s3_sa_wo_ln"), \
                    tc.tile_pool(name="ps_s3", bufs=1, space="PSUM") as ps3:
                fillers = []
                kcbs = []
                for h in range(H):
                    kcbs += kh_proj_cbs(enc_xt, "wk_ca", "bk_ca", h, ps3,
                                        kh_ca)
                vcbs = v_proj_cbs(enc_xt, "wv_ca", "bv_ca", vcat_ca, ps3)
                fillers = vcbs[:MT] + _weave(kcbs, vcbs[MT:])
                wo_ln_block(ycat, wots_sa, "bo_sa", dram["dec_nat"], F32,
                            "g1", "b1", x1_store, x1t, ps3, fillers, "a")
        enc_ctx.__exit__(None, None, None)
        xt = x1t

        # ================= S4: cross-attention units =================
        wots_ca, wo_ca_cbs = load_wo("wo_ca")
        with tc.tile_pool(name="ya_ca", bufs=1) as yap:
            ycat = yap.tile([P, H, K], BF16, name="ycat_ca", tag="ycat")
            with nc.named_scope("s4_ca_att"), \
                    tc.tile_pool(name="ps_s4", bufs=1, space="PSUM") as ps4:
                attention(xt, kh_ca, vcat_ca, ycat, "wq_ca", "bq_ca", ps4,
                          wo_ca_cbs)

            # ===== S5: ca Wo+LN; x2 tiles stay resident in SBUF for FFN =====
            x2t = new_xt("x2T")
            x2r_tiles = []
            with nc.named_scope("s5_ca_wo_ln"), \
                    tc.tile_pool(name="ps_s5", bufs=1, space="PSUM") as ps5:
                wo_ln_block(ycat, wots_ca, "bo_ca", x1_store, BF16,
                            "g2", "b2", None, x2t, ps5, [], "b",
                            x_out=x2r_tiles)
        xt = x2t
        wo_ctx.__exit__(None, None, None)
        vcat_ctx.__exit__(None, None, None)
        kh_ctx.__exit__(None, None, None)

        # ================= S6: feed-forward =================
        with tc.tile_pool(name="rtp", bufs=1) as rtp, \
                nc.named_scope("s6_ffn"):
            rt = rtp.tile([P, FT, K], BF16, name="rt", tag="rt")
            # FFN1: ft-outer, both query halves per ft; W1 loaded once.
            with tc.tile_pool(name="ps_f1", bufs=1, space="PSUM") as psf:
                for ft in range(FT):
                    w1t = wt.tile([P, MT, P], BF16, name=f"w1_{ft}", tag="wt")
                    nc.sync.dma_start(out=w1t, in_=dram["w1"][ft].rearrange(
                        "p (mt d) -> p mt d", mt=MT))
                    for kqh in range(2):
                        pf = psf.tile([P, QW], F32, name=f"pf{ft}_{kqh}",
                                      tag="pf", bufs=3)
                        for mt in range(MT):
                            nc.tensor.matmul(
                                pf, w1t[:, mt, :],
                                xt[:, mt, kqh * QW:(kqh + 1) * QW],
                                start=(mt == 0), stop=(mt == MT - 1))
                        dst = rt[:, ft, kqh * QW:(kqh + 1) * QW]
                        if flags["bf1"]:
                            nc.scalar.activation(
                                dst, pf, AF.Relu,
                                bias=bias_tiles["bf1"][:, ft:ft + 1])
                        else:
                            nc.scalar.activation(dst, pf, AF.Relu)
            # FFN2: kt-groups of 4, W2 streamed per (group, mh); stats of the
            # first z-half computed during the second half's matmuls, LN of
            # group 0 overlaps group 1's matmuls.
            with tc.tile_pool(name="ps_f2", bufs=1, space="PSUM") as psf2, \
                    tc.tile_pool(name="w2s", bufs=6) as w2sp:
                pending_ln = []
                for ktg in range(2):
                    z3s, stats_s, paccs = {}, {}, {}
                    for mh in range(2):
                        for ft in range(FT):
                            w2t = w2sp.tile([P, QW], BF16,
                                            name=f"w2_{ktg}_{mh}_{ft}",
                                            tag="w2s", bufs=6)
                            nc.sync.dma_start(
                                out=w2t,
                                in_=dram["w2"][ft, :, mh * QW:(mh + 1) * QW])
                            for ks in range(4):
                                kt = ktg * 4 + ks
                                if ft == 0:
                                    paccs[ks] = psf2.tile(
                                        [P, QW], F32, name=f"po{kt}_{mh}",
                                        tag="pacc", bufs=8)
                                nc.tensor.matmul(
                                    paccs[ks],
                                    rt[:, ft, kt * P:(kt + 1) * P], w2t,
                                    start=(ft == 0), stop=(ft == FT - 1))
                            if ft % 8 == 7 and pending_ln:
                                pending_ln.pop(0)()
                        sl = slice(mh * QW, (mh + 1) * QW)
                        for ks in range(4):
                            kt = ktg * 4 + ks
                            if mh == 0:
                                z3s[ks] = natp.tile([P, M], F32,
                                                    name=f"z3_{kt}",
                                                    tag="z", bufs=4)
                                stats_s[ks] = statp.tile(
                                    [P, 2, 6], F32, name=f"stc{kt}",
                                    tag="stats")
                            z3 = z3s[ks]
                            if flags["bf2"]:
                                nc.vector.scalar_tensor_tensor(
                                    out=z3[:, sl], in0=paccs[ks], scalar=1.0,
                                    in1=bias_tiles["bf2"][:, sl],
                                    op0=OP.bypass, op1=OP.add)
                                nc.vector.tensor_add(z3[:, sl], z3[:, sl],
                                                     x2r_tiles[kt][:, sl])
                            else:
                                nc.vector.tensor_add(z3[:, sl], paccs[ks],
                                                     x2r_tiles[kt][:, sl])
                            if mh == 0:
                                nc.vector.bn_stats(out=stats_s[ks][:, 0, :],
                                                   in_=z3[:, sl])
                    def ln_cb(ks, z3s=z3s, stats_s=stats_s, ktg=ktg):
                        def cb():
                            ln_tail(z3s[ks], ktg * 4 + ks, "g3", "b3", None,
                                    True, natp, "c", stats=stats_s[ks],
                                    have_sg=1)
                        return cb
                    pending_ln += [ln_cb(ks) for ks in range(4)]
                for cb in pending_ln:
                    cb()


def _pack_inputs(inputs: dict):
    """Host-side packing -> (flags, per-core in_maps)."""
    f32 = np.float32
    bf = ml_dtypes.bfloat16
    dec = np.asarray(inputs["dec"], f32)
    enc = np.asarray(inputs["enc"], f32)

    def nz(x):
        return bool(np.any(np.asarray(x) != 0.0))

    flags = {
        "bq_sa": nz(inputs["bq_sa"]), "bk_sa": nz(inputs["bk_sa"]),
        "bv_sa": nz(inputs["bv_sa"]), "bo_sa": nz(inputs["bo_sa"]),
        "bq_ca": nz(inputs["bq_ca"]), "bk_ca": nz(inputs["bk_ca"]),
        "bv_ca": nz(inputs["bv_ca"]), "bo_ca": nz(inputs["bo_ca"]),
        "bf1": nz(inputs["bf1"]), "bf2": nz(inputs["bf2"]),
        "g1": bool(np.any(np.asarray(inputs["g1"]) != 1.0)),
        "b1": nz(inputs["b1"]),
        "g2": bool(np.any(np.asarray(inputs["g2"]) != 1.0)),
        "b2": nz(inputs["b2"]),
        "g3": bool(np.any(np.asarray(inputs["g3"]) != 1.0)),
        "b3": nz(inputs["b3"]),
    }

    def qk_pack(w):
        w = np.asarray(w, f32)  # (H, DH, M)
        return (w.transpose(0, 2, 1).reshape(H, MT, P, DH)
                .transpose(0, 2, 1, 3).reshape(H, P, MT * DH)).astype(bf)

    def v_pack(w):
        w = np.asarray(w, f32)  # (H, DH, M) -> WvT [m, hd]
        wt_ = w.transpose(2, 0, 1).reshape(M, HD)
        return wt_.reshape(MT, P, HD).astype(bf)

    def o_pack(w):  # (M, HD) -> WoT (HD, M) -> (HT, P, M)
        return (np.ascontiguousarray(np.asarray(w, f32).T)
                .reshape(HT, P, M).astype(bf))

    W1 = np.asarray(inputs["W1"], f32)
    W2 = np.asarray(inputs["W2"], f32)
    shared = {
        "wq_sa": qk_pack(inputs["Wq_sa"]), "wk_sa": qk_pack(inputs["Wk_sa"]),
        "wv_sa": v_pack(inputs["Wv_sa"]), "wo_sa": o_pack(inputs["Wo_sa"]),
        "wq_ca": qk_pack(inputs["Wq_ca"]), "wk_ca": qk_pack(inputs["Wk_ca"]),
        "wv_ca": v_pack(inputs["Wv_ca"]), "wo_ca": o_pack(inputs["Wo_ca"]),
        "w1": (W1.reshape(FT, P, MT, P).transpose(0, 3, 2, 1)
               .reshape(FT, P, MT * P)).astype(bf),
        "w2": np.ascontiguousarray(W2.T).reshape(FT, P, M).astype(bf),
        "ones": np.ones(P, bf),
    }
    for nm in ("bq_sa", "bk_sa", "bq_ca", "bk_ca"):
        if flags[nm]:
            shared[nm] = np.ascontiguousarray(np.asarray(inputs[nm], f32).T)
    for nm in ("bv_sa", "bv_ca"):
        if flags[nm]:
            shared[nm] = np.asarray(inputs[nm], f32).reshape(HD)
    for nm in ("bo_sa", "bo_ca", "bf2", "g1", "b1", "g2", "b2", "g3", "b3"):
        if flags[nm]:
            shared[nm] = np.asarray(inputs[nm], f32)
    if flags["bf1"]:
        shared["bf1"] = np.ascontiguousarray(
            np.asarray(inputs["bf1"], f32).reshape(FT, P).T)

    def xt_pack(x):  # (K, M) -> transposed, partition-contiguous (P, MT*K)
        return (x.T.reshape(MT, P, K).transpose(1, 0, 2)
                .reshape(P, MT * K)).astype(bf)

    in_maps = []
    for c in range(N_CORES):
        m = dict(shared)
        m["xt_dec"] = xt_pack(dec[c])
        m["xt_enc"] = xt_pack(enc[c])
        m["dec_nat"] = np.ascontiguousarray(dec[c])
        in_maps.append(m)
    return flags, in_maps


_NC_CACHE: dict = {}


def kernel(**inputs) -> np.ndarray:
    flags, in_maps = _pack_inputs(inputs)
    key = tuple(sorted(flags.items()))
    if key not in _NC_CACHE:
        _NC_CACHE[key] = build_kernel(flags)
    nc = _NC_CACHE[key]
    res = run_bass_kernel_spmd(nc, in_maps, core_ids=list(range(N_CORES)))
    return np.stack([res.results[c]["out"] for c in range(N_CORES)])
